# revision 8
# baseline (speedup 1.0000x reference)
"""CosineAlignLoss Trainium2 kernel.

Data-parallel over batch: core b processes batch b (B=8, one NeuronCore each).
Host precomputes all mask-derived index/validity tensors (they only depend on
s_mask/t_mask, which are tiny); the device does the heavy gather + reductions.

Math restructuring (validated against the jax reference):
  huber_sum(d) = 0.5*sum(d^2) - 0.5*sum(relu(|d|-1)^2)
  ||aligned||^2 = sum(d^2) + 2*(aligned.teach) - ||teach||^2,  d = aligned-teach
  aligned.teach = sum_w weights_w * (win_w . teach)
so the device only needs, per (t-block of 128):
  7 fused dots (DVE), 7 square-accum norms (ACT), tiny softmax ops,
  7 diagonal matmuls (PE) for the weighted window sum, and 4 ACT passes
  for huber. All masking is folded into host-provided tensors.
"""

import os
import sys

import numpy as np

sys.path.insert(0, "/opt/trn_rl_repo")
os.environ.setdefault("MYCRO_LOCAL_CACHE", "1")

B, S, T, D = 8, 2048, 1024, 2048
WINDOW = 3
WIN_LEN = 7
TBLK = 128
NBLK = T // TBLK
HUBER_W = 0.4
COS_W = 0.3

LAST_RESULT = None  # BassKernelResults of the most recent run (for test harness)


def _host_precompute(s_mask, t_mask):
    n_s = (s_mask == 1).sum(-1).astype(np.int64)  # [B]
    n_t = (t_mask == 1).sum(-1).astype(np.int64)
    n_t_safe = np.maximum(n_t, 1)
    t_pos = np.arange(T, dtype=np.int64)[None, :]
    s_pos = (t_pos * n_s[:, None]) // n_t_safe[:, None]  # [B, T]
    w_start = np.maximum(s_pos - WINDOW, 0)
    w_end = np.minimum(n_s[:, None], s_pos + WINDOW + 1)
    j = w_start[:, :, None] + np.arange(WIN_LEN)[None, None, :]  # [B,T,7]
    j_valid = j < w_end[:, :, None]
    jc = np.clip(j, 0, S - 1).astype(np.int32)
    valid = (t_mask == 1) & (n_t[:, None] > 0) & (n_s[:, None] > 0) & (w_start < w_end)
    m3 = (3.0 * j_valid).astype(np.float32)
    badd = np.where(j_valid, 0.0, -1e4).astype(np.float32)
    vf = valid.astype(np.float32)[:, :, None]  # [B,T,1]
    return jc, m3, badd, vf, valid


def _split_multi_waits(bir: dict) -> dict:
    """Walrus in this container accepts one sync-wait per instruction.

    Tile emits several; split the extras onto standalone EventSemaphore
    instructions (wait-only) inserted immediately before, on the same engine,
    which its sequencer processes in order — semantics are identical.
    """
    ctr = 0
    for fn in bir["functions"]:
        for blk in fn["blocks"]:
            out = []
            for inst in blk["instructions"]:
                si = inst.get("sync_info")
                waits = (si or {}).get("on_wait") or []
                if len(waits) > 1:
                    for w in waits[:-1]:
                        out.append(
                            {
                                "debug": inst.get("debug", 0),
                                "engine": inst["engine"],
                                "ins": [],
                                "outs": [],
                                "name": f"splitw_{ctr}",
                                "opcode": "EventSemaphore",
                                "sync_info": {"on_update": [], "on_wait": [w]},
                            }
                        )
                        ctr += 1
                    si["on_wait"] = [waits[-1]]
                out.append(inst)
            blk["instructions"] = out
    return bir


def _install_bir_postpass():
    import json

    import concourse.bass as bass

    if getattr(bass.Bass, "_split_waits_hooked", False):
        return
    orig = bass.Bass.to_json_bytes

    def to_json_bytes(self):
        data = json.loads(orig(self))
        return json.dumps(_split_multi_waits(data)).encode()

    bass.Bass.to_json_bytes = to_json_bytes
    bass.Bass._split_waits_hooked = True


def _build_bass():
    import concourse.bass as bass
    import concourse.tile as tile
    from concourse import mybir
    from concourse.bass import IndirectOffsetOnAxis
    from concourse.masks import make_identity
    from contextlib import ExitStack

    f32 = mybir.dt.float32
    i32 = mybir.dt.int32
    ALU = mybir.AluOpType
    ACT = mybir.ActivationFunctionType

    nc = bass.Bass("TRN2", target_bir_lowering=False, debug=False)

    student = nc.dram_tensor("student", [S, D], f32, kind="ExternalInput").ap()
    teacher = nc.dram_tensor("teacher", [T, D], f32, kind="ExternalInput").ap()
    jc_d = nc.dram_tensor("jc", [T, WIN_LEN], i32, kind="ExternalInput").ap()
    m3_d = nc.dram_tensor("m3", [T, WIN_LEN], f32, kind="ExternalInput").ap()
    badd_d = nc.dram_tensor("badd", [T, WIN_LEN], f32, kind="ExternalInput").ap()
    vf_d = nc.dram_tensor("vf", [T, 1], f32, kind="ExternalInput").ap()
    out_h = nc.dram_tensor("out_h", [TBLK, 1], f32, kind="ExternalOutput").ap()
    out_c = nc.dram_tensor("out_c", [TBLK, 1], f32, kind="ExternalOutput").ap()

    with tile.TileContext(nc) as tc, ExitStack() as ctx:
        singles = ctx.enter_context(tc.tile_pool(name="singles", bufs=1))
        wins_p = ctx.enter_context(tc.tile_pool(name="wins", bufs=2))
        scr_p = ctx.enter_context(tc.tile_pool(name="scr", bufs=1))
        small = ctx.enter_context(tc.tile_pool(name="small", bufs=2))
        diag_p = ctx.enter_context(tc.tile_pool(name="diag", bufs=2))
        psum_p = ctx.enter_context(tc.tile_pool(name="psum", bufs=2, space="PSUM"))

        identity = singles.tile([TBLK, TBLK], f32)
        make_identity(nc, identity[:])
        acc_h = singles.tile([TBLK, 1], f32)
        acc_c = singles.tile([TBLK, 1], f32)
        nc.vector.memset(acc_h[:], 0.0)
        nc.vector.memset(acc_c[:], 0.0)
        neg1 = singles.tile([TBLK, 1], f32)
        nc.vector.memset(neg1[:], -1.0)

        for blk in range(NBLK):
            t0 = blk * TBLK
            tsl = slice(t0, t0 + TBLK)

            teach = wins_p.tile([TBLK, D], f32, tag="teach")
            nc.sync.dma_start(out=teach[:], in_=teacher[tsl, :])
            idx = small.tile([TBLK, WIN_LEN], i32, tag="idx")
            nc.sync.dma_start(out=idx[:], in_=jc_d[tsl, :])
            m3b = small.tile([TBLK, WIN_LEN], f32, tag="m3b")
            nc.sync.dma_start(out=m3b[:], in_=m3_d[tsl, :])
            baddb = small.tile([TBLK, WIN_LEN], f32, tag="baddb")
            nc.sync.dma_start(out=baddb[:], in_=badd_d[tsl, :])
            vfb = small.tile([TBLK, 1], f32, tag="vfb")
            nc.sync.dma_start(out=vfb[:], in_=vf_d[tsl, :])

            wins = []
            for w in range(WIN_LEN):
                win_w = wins_p.tile([TBLK, D], f32, tag=f"win{w}")
                nc.gpsimd.indirect_dma_start(
                    out=win_w[:],
                    out_offset=None,
                    in_=student[:, :],
                    in_offset=IndirectOffsetOnAxis(ap=idx[:, w : w + 1], axis=0),
                )
                wins.append(win_w)

            scr_dve = scr_p.tile([TBLK, D], f32, tag="scr_dve")
            scr_act = scr_p.tile([TBLK, D], f32, tag="scr_act")
            scr_act2 = scr_p.tile([TBLK, D], f32, tag="scr_act2")

            dots = small.tile([TBLK, WIN_LEN], f32, tag="dots")
            qs = small.tile([TBLK, WIN_LEN], f32, tag="qs")
            tnq = small.tile([TBLK, 1], f32, tag="tnq")

            for w in range(WIN_LEN):
                nc.vector.scalar_tensor_tensor(
                    out=scr_dve[:],
                    in0=wins[w][:],
                    scalar=0.0,
                    in1=teach[:],
                    op0=ALU.bypass,
                    op1=ALU.mult,
                    accum_out=dots[:, w : w + 1],
                )
                nc.scalar.activation(
                    out=scr_act[:],
                    in_=wins[w][:],
                    func=ACT.Square,
                    accum_out=qs[:, w : w + 1],
                )
            nc.scalar.activation(
                out=scr_act2[:], in_=teach[:], func=ACT.Square, accum_out=tnq[:]
            )

            # 1/max(||win||,eps) and 1/max(||teach||,eps) (eps=1e-8 => clamp sq at 1e-16)
            qm = small.tile([TBLK, WIN_LEN], f32, tag="qm")
            nc.vector.tensor_scalar_max(qm[:], qs[:], 1e-16)
            sqw = small.tile([TBLK, WIN_LEN], f32, tag="sqw")
            nc.scalar.activation(out=sqw[:], in_=qm[:], func=ACT.Sqrt)
            invw = small.tile([TBLK, WIN_LEN], f32, tag="invw")
            nc.vector.reciprocal(out=invw[:], in_=sqw[:])

            tnm = small.tile([TBLK, 1], f32, tag="tnm")
            nc.vector.tensor_scalar_max(tnm[:], tnq[:], 1e-16)
            stn = small.tile([TBLK, 1], f32, tag="stn")
            nc.scalar.activation(out=stn[:], in_=tnm[:], func=ACT.Sqrt)
            invt = small.tile([TBLK, 1], f32, tag="invt")
            nc.vector.reciprocal(out=invt[:], in_=stn[:])

            # logits = (dots*invt)*invw * m3 + badd   (invalid slots -> -1e4 -> exp==0)
            e1 = small.tile([TBLK, WIN_LEN], f32, tag="e1")
            nc.vector.scalar_tensor_tensor(
                out=e1[:], in0=dots[:], scalar=invt[:], in1=invw[:],
                op0=ALU.mult, op1=ALU.mult,
            )
            e2 = small.tile([TBLK, WIN_LEN], f32, tag="e2")
            nc.vector.tensor_tensor(out=e2[:], in0=e1[:], in1=m3b[:], op=ALU.mult)
            logits = small.tile([TBLK, WIN_LEN], f32, tag="logits")
            nc.vector.tensor_tensor(out=logits[:], in0=e2[:], in1=baddb[:], op=ALU.add)

            exps = small.tile([TBLK, WIN_LEN], f32, tag="exps")
            sumexp = small.tile([TBLK, 1], f32, tag="sumexp")
            nc.scalar.activation(
                out=exps[:], in_=logits[:], func=ACT.Exp, accum_out=sumexp[:]
            )
            sumc = small.tile([TBLK, 1], f32, tag="sumc")
            nc.vector.tensor_scalar_max(sumc[:], sumexp[:], 1e-30)
            rsum = small.tile([TBLK, 1], f32, tag="rsum")
            nc.vector.reciprocal(out=rsum[:], in_=sumc[:])
            weights = small.tile([TBLK, WIN_LEN], f32, tag="weights")
            nc.vector.tensor_scalar_mul(weights[:], exps[:], rsum[:])

            # at = aligned . teach = sum_w weights*dots
            scr7 = small.tile([TBLK, WIN_LEN], f32, tag="scr7")
            at = small.tile([TBLK, 1], f32, tag="at")
            nc.vector.scalar_tensor_tensor(
                out=scr7[:], in0=weights[:], scalar=0.0, in1=dots[:],
                op0=ALU.bypass, op1=ALU.mult, accum_out=at[:],
            )

            # aligned = sum_w weights_w * win_w via diagonal matmuls on PE
            psum_al = psum_p.tile([TBLK, D], f32, space="PSUM", tag="psum_al")
            for w in range(WIN_LEN):
                dg = diag_p.tile([TBLK, TBLK], f32, tag=f"dg{w}")
                nc.vector.tensor_scalar_mul(dg[:], identity[:], weights[:, w : w + 1])
                for n in range(D // 512):
                    nc.tensor.matmul(
                        out=psum_al[:, n * 512 : (n + 1) * 512],
                        lhsT=dg[:],
                        rhs=wins[w][:, n * 512 : (n + 1) * 512],
                        start=(w == 0),
                        stop=(w == WIN_LEN - 1),
                    )

            d_t = scr_p.tile([TBLK, D], f32, tag="d_t")
            nc.vector.tensor_tensor(
                out=d_t[:], in0=psum_al[:], in1=teach[:], op=ALU.subtract
            )

            ddot = small.tile([TBLK, 1], f32, tag="ddot")
            nc.scalar.activation(
                out=scr_act[:], in_=d_t[:], func=ACT.Square, accum_out=ddot[:]
            )
            dabs = scr_p.tile([TBLK, D], f32, tag="dabs")
            nc.scalar.activation(out=dabs[:], in_=d_t[:], func=ACT.Abs)
            nc.scalar.activation(out=scr_act2[:], in_=dabs[:], func=ACT.Relu, bias=neg1[:])
            r2sum = small.tile([TBLK, 1], f32, tag="r2sum")
            nc.scalar.activation(
                out=scr_act[:], in_=scr_act2[:], func=ACT.Square, accum_out=r2sum[:]
            )

            # hub_tok = (0.5*ddot - 0.5*r2sum)/D
            hub1 = small.tile([TBLK, 1], f32, tag="hub1")
            nc.vector.tensor_tensor(out=hub1[:], in0=ddot[:], in1=r2sum[:], op=ALU.subtract)
            hub_tok = small.tile([TBLK, 1], f32, tag="hub_tok")
            nc.vector.tensor_scalar_mul(hub_tok[:], hub1[:], 0.5 / D)

            # an_sq = ddot + 2*at - tnq ; cos_tok = 1 - at*inva*invt
            an1 = small.tile([TBLK, 1], f32, tag="an1")
            nc.vector.scalar_tensor_tensor(
                out=an1[:], in0=at[:], scalar=2.0, in1=ddot[:], op0=ALU.mult, op1=ALU.add
            )
            an2 = small.tile([TBLK, 1], f32, tag="an2")
            nc.vector.tensor_tensor(out=an2[:], in0=an1[:], in1=tnq[:], op=ALU.subtract)
            an3 = small.tile([TBLK, 1], f32, tag="an3")
            nc.vector.tensor_scalar_max(an3[:], an2[:], 1e-16)
            sa = small.tile([TBLK, 1], f32, tag="sa")
            nc.scalar.activation(out=sa[:], in_=an3[:], func=ACT.Sqrt)
            inva = small.tile([TBLK, 1], f32, tag="inva")
            nc.vector.reciprocal(out=inva[:], in_=sa[:])
            c1 = small.tile([TBLK, 1], f32, tag="c1")
            nc.vector.tensor_tensor(out=c1[:], in0=at[:], in1=inva[:], op=ALU.mult)
            c2 = small.tile([TBLK, 1], f32, tag="c2")
            nc.vector.tensor_tensor(out=c2[:], in0=c1[:], in1=invt[:], op=ALU.mult)
            cos_tok = small.tile([TBLK, 1], f32, tag="cos_tok")
            nc.vector.tensor_scalar(
                out=cos_tok[:], in0=c2[:], scalar1=-1.0, scalar2=1.0,
                op0=ALU.mult, op1=ALU.add,
            )

            # masked accumulate: acc += vf * tok
            nc.vector.scalar_tensor_tensor(
                out=acc_h[:], in0=hub_tok[:], scalar=vfb[:], in1=acc_h[:],
                op0=ALU.mult, op1=ALU.add,
            )
            nc.vector.scalar_tensor_tensor(
                out=acc_c[:], in0=cos_tok[:], scalar=vfb[:], in1=acc_c[:],
                op0=ALU.mult, op1=ALU.add,
            )

        nc.sync.dma_start(out=out_h[:, :], in_=acc_h[:])
        nc.sync.dma_start(out=out_c[:, :], in_=acc_c[:])

    return nc


def kernel(student_output, teacher_output, original_student, s_mask, t_mask):
    global LAST_RESULT
    from concourse.bass_utils import run_bass_kernel_spmd

    _install_bir_postpass()

    student_output = np.ascontiguousarray(np.asarray(student_output, dtype=np.float32))
    teacher_output = np.ascontiguousarray(np.asarray(teacher_output, dtype=np.float32))
    s_mask = np.asarray(s_mask)
    t_mask = np.asarray(t_mask)

    jc, m3, badd, vf, valid = _host_precompute(s_mask, t_mask)

    nc = _build_bass()
    in_maps = [
        {
            "student": student_output[b],
            "teacher": teacher_output[b],
            "jc": jc[b],
            "m3": m3[b],
            "badd": badd[b],
            "vf": vf[b],
        }
        for b in range(B)
    ]
    trace = bool(int(os.environ.get("KERNEL_TRACE", "0")))
    tmpdir = os.environ.get("KERNEL_TMPDIR")
    res = run_bass_kernel_spmd(
        nc, in_maps, core_ids=list(range(B)), trace=trace, tmpdir=tmpdir
    )
    LAST_RESULT = res

    hsum = 0.0
    csum = 0.0
    for r in res.results:
        hsum += float(np.asarray(r["out_h"], dtype=np.float64).sum())
        csum += float(np.asarray(r["out_c"], dtype=np.float64).sum())

    count = int(valid.sum())
    cnt_safe = max(count, 1)
    if count > 0:
        huber_mean = hsum / cnt_safe
        cos_mean = csum / cnt_safe
        token_loss = HUBER_W * huber_mean + COS_W * cos_mean
    else:
        huber_mean = cos_mean = token_loss = 0.0
    return (
        np.float32(token_loss),
        np.float32(huber_mean),
        np.float32(cos_mean),
        np.int32(count),
    )
